# revision 5
# baseline (speedup 1.0000x reference)
"""GCNCheb (K=4) distributed Trainium2 kernel.

out = sum_k T_k(L) x @ W_k + bias  with  T0=x, T1=Lx, T2=2L T1 - T0, T3=2L T2 - T1.

Folded form computed here (host folds weights):
    y1 = L x, y2 = L y1, y3 = L y2
    out = x@(W0-W2) + y1@(W1-3W3) + y2@(2W2) + y3@(4W3) + bias

Sharding: rows (and edges by destination row) across 8 NeuronCores; x replicated;
y1,y2 all-gathered (bf16) between SpMM stages; small weights replicated.

SpMM on each core: dma_gather neighbor rows (bf16) -> per-128-edge block build a
selector matrix S'[e, r] = lap_e * (r == row_rel_e) with one fused DVE op ->
PE matmul S'.T @ G accumulated in a 128-row PSUM window -> fp32 SBUF accumulator
(4 passes, one per int16 index chunk of the source) -> cast bf16, store row-major.

Dense tail: per 128-row tile, lhsT = transposed T_k tiles (DMA-transpose loads for
y1..y3, host-pretransposed hi/lo split for x), rhs = folded A_k (hi/lo split),
accumulated in PSUM fp32, + bias.
"""
import sys

if '/opt/trn_rl_repo' not in sys.path:
    sys.path.insert(0, '/opt/trn_rl_repo')

import numpy as np
import ml_dtypes

import concourse.bacc as bacc
import concourse.mybir as mybir
from concourse.tile import TileContext
from concourse.bass_utils import run_bass_kernel_spmd
from concourse.library_config import mlp

BF16 = ml_dtypes.bfloat16


class CFG:
    N = 100000          # nodes
    C = 128             # feature width (in == out)
    K = 4
    NCORES = 8
    WIN = 128           # PSUM window rows
    CHUNK = 32768       # int16 index range per gather source chunk
    GBLK = 64           # max 128-edge blocks per dma_gather call

    def __init__(self, N=100000, ncores=8, chunk=32768, gblk=8):
        self.N = N
        self.NCORES = ncores
        self.CHUNK = chunk
        self.GBLK = gblk
        self.RPC = N // ncores                      # rows per core
        self.NWIN = (self.RPC + self.WIN - 1) // self.WIN
        self.RPC_PAD = self.NWIN * self.WIN
        self.NPAD = ncores * self.RPC_PAD
        self.NCHUNK = (self.NPAD + self.CHUNK - 1) // self.CHUNK


def _prep_meta(cfg, edge_index, lap):
    """Host preprocessing -> per-core gather/selector metadata + call structure.

    Returns dict with per-core arrays and the (shared) compiled-in structure.
    """
    C = cfg.C
    row = np.asarray(edge_index[0], dtype=np.int64)
    col = np.asarray(edge_index[1], dtype=np.int64)
    lap = np.asarray(lap, dtype=np.float32)

    core = row // cfg.RPC
    r_loc = row - core * cfg.RPC
    w = r_loc // cfg.WIN
    rel = (r_loc % cfg.WIN).astype(np.int32)

    ccore = col // cfg.RPC
    p_col = ccore * cfg.RPC_PAD + (col - ccore * cfg.RPC)
    chunk = p_col // cfg.CHUNK
    cidx = (p_col % cfg.CHUNK).astype(np.int32)

    NW, NC_, NCH = cfg.NWIN, cfg.NCORES, cfg.NCHUNK
    gid = ((core * NCH + chunk) * NW + w).astype(np.int64)
    counts = np.bincount(gid, minlength=NC_ * NCH * NW).reshape(NC_, NCH, NW)
    # common (all-core) block-count grid
    nb_cw = (counts.max(axis=0) + 127) // 128          # [NCH, NW]
    cap_cw = nb_cw * 128
    # base offset of each (chunk, window) run inside the padded per-core stream
    base_cw = np.zeros((NCH, NW), np.int64)
    flat = cap_cw.reshape(-1)
    base_cw.reshape(-1)[1:] = np.cumsum(flat)[:-1]
    tot_idx = int(flat.sum())
    nblk = tot_idx // 128

    # rank of each edge within its (core, chunk, window) group
    order = np.argsort(gid, kind='stable')
    gsorted = gid[order]
    starts = np.searchsorted(gsorted, np.arange(NC_ * NCH * NW))
    ranks = np.empty(len(gid), np.int64)
    ranks[order] = np.arange(len(gid)) - starts[gsorted]
    pos = base_cw[chunk, w] + ranks                     # position in core's stream

    idx16 = np.zeros((NC_, tot_idx), np.int16)
    lap_pad = np.zeros((NC_, tot_idx), np.float32)
    rel_pad = np.zeros((NC_, tot_idx), np.int32)
    idx16[core, pos] = cidx
    lap_pad[core, pos] = lap
    rel_pad[core, pos] = rel

    # per-block window id + call structure (shared across cores)
    blk_win = np.repeat(
        np.tile(np.arange(NW), NCH), nb_cw.reshape(-1)
    )  # [nblk] window of each block, chunk-major
    blk_chunk = np.repeat(np.arange(NCH), nb_cw.sum(axis=1))
    calls = []          # (chunk, blk0, nblk_call)
    b = 0
    for c in range(NCH):
        nb_c = int(nb_cw[c].sum())
        while nb_c > 0:
            take = min(cfg.GBLK, nb_c)
            calls.append((c, b, take))
            b += take
            nb_c -= take
    assert b == nblk

    # wrapped + replicated idx layout per call: [128, tot_idx // 16]
    idx_w = np.empty((NC_, 128, tot_idx // 16), np.int16)
    for (c, blk0, nb) in calls:
        p0, n = blk0 * 128, nb * 128
        seg = idx16[:, p0:p0 + n].reshape(NC_, n // 16, 16).transpose(0, 2, 1)
        idx_w[:, :, p0 // 16:(p0 + n) // 16] = np.tile(seg, (1, 8, 1))

    rel_t = rel_pad.reshape(NC_, nblk, 128).transpose(0, 2, 1).astype(np.float32)
    lap_t = lap_pad.reshape(NC_, nblk, 128).transpose(0, 2, 1).astype(np.float32)

    return dict(
        nblk=nblk, tot_idx=tot_idx, calls=calls,
        blk_win=blk_win, blk_chunk=blk_chunk,
        idx_w=idx_w, rel_t=rel_t, lap_t=lap_t,
    )


def _build_nc(cfg, meta):
    C = cfg.C
    nblk, tot_idx, calls, blk_win = (
        meta['nblk'], meta['tot_idx'], meta['calls'], meta['blk_win'])

    nc = bacc.Bacc("TRN2", num_devices=cfg.NCORES, num_swdge_queues=4)
    f32, bf16, i16 = mybir.dt.float32, mybir.dt.bfloat16, mybir.dt.int16

    x_pad = nc.dram_tensor("x_pad", [cfg.NPAD, C], bf16, kind="ExternalInput")
    idx_hbm = nc.dram_tensor("idx", [128, tot_idx // 16], i16, kind="ExternalInput")
    rel_hbm = nc.dram_tensor("rel", [128, nblk], f32, kind="ExternalInput")
    lap_hbm = nc.dram_tensor("lapm", [128, nblk], f32, kind="ExternalInput")
    iota_hbm = nc.dram_tensor("iota", [128, C], bf16, kind="ExternalInput")
    aw_hbm = nc.dram_tensor("aw", [128, 8 * C], bf16, kind="ExternalInput")
    bias_hbm = nc.dram_tensor("biasr", [128, C], f32, kind="ExternalInput")
    xT_hi_hbm = nc.dram_tensor("xT_hi", [128, cfg.RPC_PAD], bf16, kind="ExternalInput")
    xT_lo_hbm = nc.dram_tensor("xT_lo", [128, cfg.RPC_PAD], bf16, kind="ExternalInput")
    out_loc = nc.dram_tensor("out_loc", [cfg.RPC, C], f32, kind="ExternalOutput")

    y_loc = [nc.dram_tensor(f"y{s}_loc", [cfg.RPC_PAD, C], bf16) for s in range(3)]
    y_full = [
        nc.dram_tensor(f"y{s}_full", [cfg.NPAD, C], bf16, addr_space="Shared")
        for s in range(2)
    ]
    rg = [list(range(cfg.NCORES))]

    with TileContext(nc) as tc:
        nc.gpsimd.load_library(mlp)
        with tc.tile_pool(name="const", bufs=1) as constp, \
             tc.tile_pool(name="meta", bufs=1) as metap, \
             tc.tile_pool(name="acc", bufs=1) as accp, \
             tc.tile_pool(name="idxp", bufs=8) as idxp, \
             tc.tile_pool(name="gat", bufs=8) as gatp, \
             tc.tile_pool(name="sp", bufs=6) as spp, \
             tc.tile_pool(name="fl", bufs=3) as flp, \
             tc.tile_pool(name="tl", bufs=3) as tlp, \
             tc.tile_pool(name="ps", bufs=2, space="PSUM") as psp, \
             tc.tile_pool(name="pso", bufs=2, space="PSUM") as psop:

            iota_t = constp.tile([128, C], bf16)
            nc.sync.dma_start(iota_t[:], iota_hbm[:])
            aw_t = constp.tile([128, 8 * C], bf16)
            nc.sync.dma_start(aw_t[:], aw_hbm[:])
            bias_t = constp.tile([128, C], f32)
            nc.sync.dma_start(bias_t[:], bias_hbm[:])
            rel_t = metap.tile([128, nblk], f32)
            nc.sync.dma_start(rel_t[:], rel_hbm[:])
            lap_t = metap.tile([128, nblk], f32)
            nc.sync.dma_start(lap_t[:], lap_hbm[:])

            y_acc = accp.tile([128, cfg.NWIN * 128], f32)

            for s in range(3):
                src = x_pad if s == 0 else y_full[s - 1]
                nc.vector.memset(y_acc[:], 0.0)
                psum_t = None
                cur_key = None
                for ci, (c, blk0, nbc) in enumerate(calls):
                    n = nbc * 128
                    idx_t = idxp.tile([128, cfg.GBLK * 8], i16)
                    nc.sync.dma_start(
                        idx_t[:, :n // 16],
                        idx_hbm[:, blk0 * 8:blk0 * 8 + n // 16])
                    g = gatp.tile([128, cfg.GBLK, C], bf16)
                    c_rows = min(cfg.CHUNK, cfg.NPAD - c * cfg.CHUNK)
                    nc.gpsimd.dma_gather(
                        g[:, :nbc, :],
                        src[c * cfg.CHUNK:c * cfg.CHUNK + c_rows, :],
                        idx_t[:, :n // 16], n, n, C, queue_num=ci % 4)
                    for j in range(nbc):
                        b = blk0 + j
                        w = int(blk_win[b])
                        key = (c, w)
                        if key != cur_key:
                            # flush previous window psum into the accumulator
                            if psum_t is not None:
                                pw = cur_key[1]
                                nc.vector.tensor_add(
                                    y_acc[:, pw * 128:(pw + 1) * 128],
                                    y_acc[:, pw * 128:(pw + 1) * 128],
                                    psum_t[:])
                            psum_t = psp.tile([128, C], f32)
                            cur_key = key
                            first = True
                        else:
                            first = False
                        nxt_key = (
                            (int(meta['blk_chunk'][b + 1]), int(blk_win[b + 1]))
                            if b + 1 < nblk else None)
                        last = (nxt_key != key)
                        S = spp.tile([128, C], bf16)
                        nc.vector.tensor_scalar(
                            S[:], iota_t[:], rel_t[:, b:b + 1], lap_t[:, b:b + 1],
                            mybir.AluOpType.is_equal, mybir.AluOpType.mult)
                        nc.tensor.matmul(
                            psum_t[:], lhsT=S[:], rhs=g[:, j, :],
                            start=first, stop=last)
                if psum_t is not None:
                    pw = cur_key[1]
                    nc.vector.tensor_add(
                        y_acc[:, pw * 128:(pw + 1) * 128],
                        y_acc[:, pw * 128:(pw + 1) * 128],
                        psum_t[:])
                    psum_t = None
                    cur_key = None
                # cast + store y windows (row-major bf16)
                for wi in range(cfg.NWIN):
                    yb = flp.tile([128, C], bf16)
                    nc.vector.tensor_copy(yb[:], y_acc[:, wi * 128:(wi + 1) * 128])
                    nc.sync.dma_start(y_loc[s][wi * 128:(wi + 1) * 128, :], yb[:])
                if s < 2:
                    nc.gpsimd.collective_compute(
                        "AllGather", mybir.AluOpType.bypass,
                        replica_groups=rg,
                        ins=[y_loc[s][:]], outs=[y_full[s][:]])

            # dense tail
            for t in range(cfg.NWIN):
                sl = slice(t * 128, (t + 1) * 128)
                xh = tlp.tile([128, 128], bf16)
                nc.sync.dma_start(xh[:], xT_hi_hbm[:, sl])
                xl = tlp.tile([128, 128], bf16)
                nc.sync.dma_start(xl[:], xT_lo_hbm[:, sl])
                y1t = tlp.tile([128, 128], bf16)
                nc.sync.dma_start(y1t[:], y_loc[0][sl, :], transpose=True)
                y2t = tlp.tile([128, 128], bf16)
                nc.sync.dma_start(y2t[:], y_loc[1][sl, :], transpose=True)
                y3t = tlp.tile([128, 128], bf16)
                nc.sync.dma_start(y3t[:], y_loc[2][sl, :], transpose=True)
                terms = [
                    (xh, 0), (xh, 1), (xl, 0),       # x @ (A0_hi + A0_lo)
                    (y1t, 2), (y1t, 3),
                    (y2t, 4), (y2t, 5),
                    (y3t, 6), (y3t, 7),
                ]
                po = psop.tile([128, C], f32)
                for i, (tt, ai) in enumerate(terms):
                    nc.tensor.matmul(
                        po[:], lhsT=tt[:], rhs=aw_t[:, ai * C:(ai + 1) * C],
                        start=(i == 0), stop=(i == len(terms) - 1))
                ot = flp.tile([128, C], f32)
                nc.vector.tensor_add(ot[:], po[:], bias_t[:])
                rows = min(128, cfg.RPC - t * 128)
                nc.sync.dma_start(out_loc[t * 128:t * 128 + rows, :], ot[:rows, :])

    nc.compile()
    return nc


def _fold_weights(weight, bias):
    W = np.asarray(weight, dtype=np.float32)
    A = np.stack([W[0] - W[2], W[1] - 3.0 * W[3], 2.0 * W[2], 4.0 * W[3]])
    aw = np.empty((128, 8 * W.shape[1]), np.float32)
    C = W.shape[1]
    for k in range(4):
        hi = A[k].astype(BF16).astype(np.float32)
        lo = A[k] - hi
        aw[:, (2 * k) * C:(2 * k + 1) * C] = hi
        aw[:, (2 * k + 1) * C:(2 * k + 2) * C] = lo
    return aw.astype(BF16), np.asarray(bias, np.float32)


_cache = {}


def _get_compiled(cfg, edge_index, lap):
    key = (cfg.N, cfg.NCORES, int(edge_index.shape[1]))
    if key not in _cache:
        meta = _prep_meta(cfg, edge_index, lap)
        nc = _build_nc(cfg, meta)
        _cache[key] = (meta, nc)
    return _cache[key]


def _run(cfg, nc, meta, x, lap, weight, bias):
    C = cfg.C
    # lap/rel metadata already in meta (built from edge_index+lap)
    x = np.asarray(x, dtype=np.float32)
    x_pad = np.zeros((cfg.NPAD, C), BF16)
    xv = x.reshape(cfg.NCORES, cfg.RPC, C)
    x_pad_v = x_pad.reshape(cfg.NCORES, cfg.RPC_PAD, C)
    x_pad_v[:, :cfg.RPC, :] = xv.astype(BF16)

    aw, bias_f = _fold_weights(weight, bias)
    bias_rep = np.tile(bias_f[None, :], (128, 1)).astype(np.float32)
    iota = np.tile(np.arange(C, dtype=np.float32)[None, :], (128, 1)).astype(BF16)

    in_maps = []
    for i in range(cfg.NCORES):
        x_loc = np.zeros((cfg.RPC_PAD, C), np.float32)
        x_loc[:cfg.RPC] = xv[i]
        xT = x_loc.T.copy()                       # [C, RPC_PAD]
        xT_hi = xT.astype(BF16)
        xT_lo = (xT - xT_hi.astype(np.float32)).astype(BF16)
        in_maps.append({
            "x_pad": x_pad,
            "idx": meta['idx_w'][i],
            "rel": meta['rel_t'][i],
            "lapm": meta['lap_t'][i],
            "iota": iota,
            "aw": aw,
            "biasr": bias_rep,
            "xT_hi": xT_hi,
            "xT_lo": xT_lo,
        })
    res = run_bass_kernel_spmd(nc, in_maps, core_ids=list(range(cfg.NCORES)))
    out = np.concatenate([res.results[i]["out_loc"] for i in range(cfg.NCORES)], axis=0)
    return out.astype(np.float32)


def kernel(x, lap, weight, bias, edge_index, num_nodes=None, **_kw):
    cfg = CFG(N=int(np.asarray(x).shape[0]), ncores=8)
    lap = np.asarray(lap, dtype=np.float32)
    edge_index = np.asarray(edge_index)
    meta, nc = _get_compiled(cfg, edge_index, lap)
    return _run(cfg, nc, meta, x, lap, weight, bias)


# revision 6
# speedup vs baseline: 3037.5627x; 3037.5627x over previous
"""GCNCheb (K=4) distributed Trainium2 kernel.

out = sum_k T_k(L) x @ W_k + bias  with  T0=x, T1=Lx, T2=2L T1 - T0, T3=2L T2 - T1.

Folded form computed here (host folds weights):
    y1 = L x, y2 = L y1, y3 = L y2
    out = x@(W0-W2) + y1@(W1-3W3) + y2@(2W2) + y3@(4W3) + bias

Sharding: rows (and edges by destination row) across 8 NeuronCores; x replicated;
y1,y2 all-gathered (bf16) between SpMM stages; small weights replicated.

SpMM on each core: dma_gather neighbor rows (bf16) -> per-128-edge block build a
selector matrix S'[e, r] = lap_e * (r == row_rel_e) with one fused DVE op ->
PE matmul S'.T @ G accumulated in a 128-row PSUM window -> fp32 SBUF accumulator
(4 passes, one per int16 index chunk of the source) -> cast bf16, store row-major.

Dense tail: per 128-row tile, lhsT = transposed T_k tiles (DMA-transpose loads for
y1..y3, host-pretransposed hi/lo split for x), rhs = folded A_k (hi/lo split),
accumulated in PSUM fp32, + bias.
"""
import sys

if '/opt/trn_rl_repo' not in sys.path:
    sys.path.insert(0, '/opt/trn_rl_repo')

import numpy as np
import ml_dtypes

import concourse.bacc as bacc
import concourse.mybir as mybir
from concourse.tile import TileContext
from concourse.bass_utils import run_bass_kernel_spmd
from concourse.library_config import mlp

BF16 = ml_dtypes.bfloat16


class CFG:
    N = 100000          # nodes
    C = 128             # feature width (in == out)
    K = 4
    NCORES = 8
    WIN = 128           # PSUM window rows
    CHUNK = 32768       # int16 index range per gather source chunk
    GBLK = 64           # max 128-edge blocks per dma_gather call

    def __init__(self, N=100000, ncores=8, chunk=32768, gblk=8):
        self.N = N
        self.NCORES = ncores
        self.CHUNK = chunk
        self.GBLK = gblk
        self.RPC = N // ncores                      # rows per core
        self.NWIN = (self.RPC + self.WIN - 1) // self.WIN
        self.RPC_PAD = self.NWIN * self.WIN
        self.NPAD = ncores * self.RPC_PAD
        self.NCHUNK = (self.NPAD + self.CHUNK - 1) // self.CHUNK


def _prep_meta(cfg, edge_index, lap):
    """Host preprocessing -> per-core gather/selector metadata + call structure.

    Returns dict with per-core arrays and the (shared) compiled-in structure.
    """
    C = cfg.C
    row = np.asarray(edge_index[0], dtype=np.int64)
    col = np.asarray(edge_index[1], dtype=np.int64)
    lap = np.asarray(lap, dtype=np.float32)

    core = row // cfg.RPC
    r_loc = row - core * cfg.RPC
    w = r_loc // cfg.WIN
    rel = (r_loc % cfg.WIN).astype(np.int32)

    ccore = col // cfg.RPC
    p_col = ccore * cfg.RPC_PAD + (col - ccore * cfg.RPC)
    chunk = p_col // cfg.CHUNK
    cidx = (p_col % cfg.CHUNK).astype(np.int32)

    NW, NC_, NCH = cfg.NWIN, cfg.NCORES, cfg.NCHUNK
    gid = ((core * NCH + chunk) * NW + w).astype(np.int64)
    counts = np.bincount(gid, minlength=NC_ * NCH * NW).reshape(NC_, NCH, NW)
    # common (all-core) block-count grid
    nb_cw = (counts.max(axis=0) + 127) // 128          # [NCH, NW]
    cap_cw = nb_cw * 128
    # base offset of each (chunk, window) run inside the padded per-core stream
    base_cw = np.zeros((NCH, NW), np.int64)
    flat = cap_cw.reshape(-1)
    base_cw.reshape(-1)[1:] = np.cumsum(flat)[:-1]
    tot_idx = int(flat.sum())
    nblk = tot_idx // 128

    # rank of each edge within its (core, chunk, window) group
    order = np.argsort(gid, kind='stable')
    gsorted = gid[order]
    starts = np.searchsorted(gsorted, np.arange(NC_ * NCH * NW))
    ranks = np.empty(len(gid), np.int64)
    ranks[order] = np.arange(len(gid)) - starts[gsorted]
    pos = base_cw[chunk, w] + ranks                     # position in core's stream

    idx16 = np.zeros((NC_, tot_idx), np.int16)
    lap_pad = np.zeros((NC_, tot_idx), np.float32)
    rel_pad = np.zeros((NC_, tot_idx), np.int32)
    idx16[core, pos] = cidx
    lap_pad[core, pos] = lap
    rel_pad[core, pos] = rel

    # per-block window id + call structure (shared across cores)
    blk_win = np.repeat(
        np.tile(np.arange(NW), NCH), nb_cw.reshape(-1)
    )  # [nblk] window of each block, chunk-major
    blk_chunk = np.repeat(np.arange(NCH), nb_cw.sum(axis=1))
    calls = []          # (chunk, blk0, nblk_call)
    b = 0
    for c in range(NCH):
        nb_c = int(nb_cw[c].sum())
        while nb_c > 0:
            take = min(cfg.GBLK, nb_c)
            calls.append((c, b, take))
            b += take
            nb_c -= take
    assert b == nblk

    # wrapped + replicated idx layout per call: [128, tot_idx // 16]
    idx_w = np.empty((NC_, 128, tot_idx // 16), np.int16)
    for (c, blk0, nb) in calls:
        p0, n = blk0 * 128, nb * 128
        seg = idx16[:, p0:p0 + n].reshape(NC_, n // 16, 16).transpose(0, 2, 1)
        idx_w[:, :, p0 // 16:(p0 + n) // 16] = np.tile(seg, (1, 8, 1))

    rel_t = rel_pad.reshape(NC_, nblk, 128).transpose(0, 2, 1).astype(np.float32)
    lap_t = lap_pad.reshape(NC_, nblk, 128).transpose(0, 2, 1).astype(np.float32)

    return dict(
        nblk=nblk, tot_idx=tot_idx, calls=calls,
        blk_win=blk_win, blk_chunk=blk_chunk,
        idx_w=idx_w, rel_t=rel_t, lap_t=lap_t,
    )


def _build_nc(cfg, meta, sim_timing=False):
    C = cfg.C
    nblk, tot_idx, calls, blk_win = (
        meta['nblk'], meta['tot_idx'], meta['calls'], meta['blk_win'])

    nc = bacc.Bacc("TRN2", num_devices=cfg.NCORES, num_swdge_queues=4)
    shared_space = "Local" if sim_timing else "Shared"  # TimelineSim: no collectives
    f32, bf16, i16 = mybir.dt.float32, mybir.dt.bfloat16, mybir.dt.int16

    x_pad = nc.dram_tensor("x_pad", [cfg.NPAD, C], bf16, kind="ExternalInput")
    idx_hbm = nc.dram_tensor("idx", [128, tot_idx // 16], i16, kind="ExternalInput")
    rel_hbm = nc.dram_tensor("rel", [128, nblk], f32, kind="ExternalInput")
    lap_hbm = nc.dram_tensor("lapm", [128, nblk], f32, kind="ExternalInput")
    iota_hbm = nc.dram_tensor("iota", [128, C], bf16, kind="ExternalInput")
    aw_hbm = nc.dram_tensor("aw", [128, 8 * C], bf16, kind="ExternalInput")
    bias_hbm = nc.dram_tensor("biasr", [128, C], f32, kind="ExternalInput")
    xT_hi_hbm = nc.dram_tensor("xT_hi", [128, cfg.RPC_PAD], bf16, kind="ExternalInput")
    xT_lo_hbm = nc.dram_tensor("xT_lo", [128, cfg.RPC_PAD], bf16, kind="ExternalInput")
    out_loc = nc.dram_tensor("out_loc", [cfg.RPC, C], f32, kind="ExternalOutput")

    y_loc = [nc.dram_tensor(f"y{s}_loc", [cfg.RPC_PAD, C], bf16) for s in range(3)]
    y_full = [
        nc.dram_tensor(f"y{s}_full", [cfg.NPAD, C], bf16, addr_space=shared_space)
        for s in range(2)
    ]
    rg = [list(range(cfg.NCORES))]

    with TileContext(nc) as tc:
        nc.gpsimd.load_library(mlp)
        with tc.tile_pool(name="const", bufs=1) as constp, \
             tc.tile_pool(name="meta", bufs=1) as metap, \
             tc.tile_pool(name="acc", bufs=1) as accp, \
             tc.tile_pool(name="idxp", bufs=8) as idxp, \
             tc.tile_pool(name="gat", bufs=8) as gatp, \
             tc.tile_pool(name="sp", bufs=6) as spp, \
             tc.tile_pool(name="fl", bufs=3) as flp, \
             tc.tile_pool(name="tl", bufs=3) as tlp, \
             tc.tile_pool(name="ps", bufs=2, space="PSUM") as psp, \
             tc.tile_pool(name="pso", bufs=2, space="PSUM") as psop:

            iota_t = constp.tile([128, C], bf16)
            nc.sync.dma_start(iota_t[:], iota_hbm[:])
            aw_t = constp.tile([128, 8 * C], bf16)
            nc.sync.dma_start(aw_t[:], aw_hbm[:])
            bias_t = constp.tile([128, C], f32)
            nc.sync.dma_start(bias_t[:], bias_hbm[:])
            rel_t = metap.tile([128, nblk], f32)
            nc.sync.dma_start(rel_t[:], rel_hbm[:])
            lap_t = metap.tile([128, nblk], f32)
            nc.sync.dma_start(lap_t[:], lap_hbm[:])

            y_acc = accp.tile([128, cfg.NWIN * 128], f32)

            for s in range(3):
                src = x_pad if s == 0 else y_full[s - 1]
                nc.vector.memset(y_acc[:], 0.0)
                psum_t = None
                cur_key = None
                for ci, (c, blk0, nbc) in enumerate(calls):
                    n = nbc * 128
                    idx_t = idxp.tile([128, cfg.GBLK * 8], i16)
                    nc.sync.dma_start(
                        idx_t[:, :n // 16],
                        idx_hbm[:, blk0 * 8:blk0 * 8 + n // 16])
                    g = gatp.tile([128, cfg.GBLK, C], bf16)
                    c_rows = min(cfg.CHUNK, cfg.NPAD - c * cfg.CHUNK)
                    nc.gpsimd.dma_gather(
                        g[:, :nbc, :],
                        src[c * cfg.CHUNK:c * cfg.CHUNK + c_rows, :],
                        idx_t[:, :n // 16], n, n, C, queue_num=ci % 4)
                    for j in range(nbc):
                        b = blk0 + j
                        w = int(blk_win[b])
                        key = (c, w)
                        if key != cur_key:
                            # flush previous window psum into the accumulator
                            if psum_t is not None:
                                pw = cur_key[1]
                                nc.vector.tensor_add(
                                    y_acc[:, pw * 128:(pw + 1) * 128],
                                    y_acc[:, pw * 128:(pw + 1) * 128],
                                    psum_t[:])
                            psum_t = psp.tile([128, C], f32)
                            cur_key = key
                            first = True
                        else:
                            first = False
                        nxt_key = (
                            (int(meta['blk_chunk'][b + 1]), int(blk_win[b + 1]))
                            if b + 1 < nblk else None)
                        last = (nxt_key != key)
                        S = spp.tile([128, C], bf16)
                        nc.vector.tensor_scalar(
                            S[:], iota_t[:], rel_t[:, b:b + 1], lap_t[:, b:b + 1],
                            mybir.AluOpType.is_equal, mybir.AluOpType.mult)
                        nc.tensor.matmul(
                            psum_t[:], lhsT=S[:], rhs=g[:, j, :],
                            start=first, stop=last)
                if psum_t is not None:
                    pw = cur_key[1]
                    nc.vector.tensor_add(
                        y_acc[:, pw * 128:(pw + 1) * 128],
                        y_acc[:, pw * 128:(pw + 1) * 128],
                        psum_t[:])
                    psum_t = None
                    cur_key = None
                # cast + store y windows (row-major bf16)
                for wi in range(cfg.NWIN):
                    yb = flp.tile([128, C], bf16)
                    nc.vector.tensor_copy(yb[:], y_acc[:, wi * 128:(wi + 1) * 128])
                    nc.sync.dma_start(y_loc[s][wi * 128:(wi + 1) * 128, :], yb[:])
                if s < 2 and not sim_timing:
                    nc.gpsimd.collective_compute(
                        "AllGather", mybir.AluOpType.bypass,
                        replica_groups=rg,
                        ins=[y_loc[s][:]], outs=[y_full[s][:]])

            # dense tail
            for t in range(cfg.NWIN):
                sl = slice(t * 128, (t + 1) * 128)
                xh = tlp.tile([128, 128], bf16)
                nc.sync.dma_start(xh[:], xT_hi_hbm[:, sl])
                xl = tlp.tile([128, 128], bf16)
                nc.sync.dma_start(xl[:], xT_lo_hbm[:, sl])
                y1t = tlp.tile([128, 128], bf16)
                nc.sync.dma_start(y1t[:], y_loc[0][sl, :], transpose=True)
                y2t = tlp.tile([128, 128], bf16)
                nc.sync.dma_start(y2t[:], y_loc[1][sl, :], transpose=True)
                y3t = tlp.tile([128, 128], bf16)
                nc.sync.dma_start(y3t[:], y_loc[2][sl, :], transpose=True)
                terms = [
                    (xh, 0), (xh, 1), (xl, 0),       # x @ (A0_hi + A0_lo)
                    (y1t, 2), (y1t, 3),
                    (y2t, 4), (y2t, 5),
                    (y3t, 6), (y3t, 7),
                ]
                po = psop.tile([128, C], f32)
                for i, (tt, ai) in enumerate(terms):
                    nc.tensor.matmul(
                        po[:], lhsT=tt[:], rhs=aw_t[:, ai * C:(ai + 1) * C],
                        start=(i == 0), stop=(i == len(terms) - 1))
                ot = flp.tile([128, C], f32)
                nc.vector.tensor_add(ot[:], po[:], bias_t[:])
                rows = min(128, cfg.RPC - t * 128)
                nc.sync.dma_start(out_loc[t * 128:t * 128 + rows, :], ot[:rows, :])

    nc.compile()
    return nc


def _fold_weights(weight, bias):
    W = np.asarray(weight, dtype=np.float32)
    A = np.stack([W[0] - W[2], W[1] - 3.0 * W[3], 2.0 * W[2], 4.0 * W[3]])
    aw = np.empty((128, 8 * W.shape[1]), np.float32)
    C = W.shape[1]
    for k in range(4):
        hi = A[k].astype(BF16).astype(np.float32)
        lo = A[k] - hi
        aw[:, (2 * k) * C:(2 * k + 1) * C] = hi
        aw[:, (2 * k + 1) * C:(2 * k + 2) * C] = lo
    return aw.astype(BF16), np.asarray(bias, np.float32)


_cache = {}


def _get_compiled(cfg, edge_index, lap):
    key = (cfg.N, cfg.NCORES, int(edge_index.shape[1]))
    if key not in _cache:
        meta = _prep_meta(cfg, edge_index, lap)
        nc = _build_nc(cfg, meta)
        _cache[key] = (meta, nc)
    return _cache[key]


def _run(cfg, nc, meta, x, lap, weight, bias):
    C = cfg.C
    # lap/rel metadata already in meta (built from edge_index+lap)
    x = np.asarray(x, dtype=np.float32)
    x_pad = np.zeros((cfg.NPAD, C), BF16)
    xv = x.reshape(cfg.NCORES, cfg.RPC, C)
    x_pad_v = x_pad.reshape(cfg.NCORES, cfg.RPC_PAD, C)
    x_pad_v[:, :cfg.RPC, :] = xv.astype(BF16)

    aw, bias_f = _fold_weights(weight, bias)
    bias_rep = np.tile(bias_f[None, :], (128, 1)).astype(np.float32)
    iota = np.tile(np.arange(C, dtype=np.float32)[None, :], (128, 1)).astype(BF16)

    in_maps = []
    for i in range(cfg.NCORES):
        x_loc = np.zeros((cfg.RPC_PAD, C), np.float32)
        x_loc[:cfg.RPC] = xv[i]
        xT = x_loc.T.copy()                       # [C, RPC_PAD]
        xT_hi = xT.astype(BF16)
        xT_lo = (xT - xT_hi.astype(np.float32)).astype(BF16)
        in_maps.append({
            "x_pad": x_pad,
            "idx": meta['idx_w'][i],
            "rel": meta['rel_t'][i],
            "lapm": meta['lap_t'][i],
            "iota": iota,
            "aw": aw,
            "biasr": bias_rep,
            "xT_hi": xT_hi,
            "xT_lo": xT_lo,
        })
    res = run_bass_kernel_spmd(nc, in_maps, core_ids=list(range(cfg.NCORES)))
    out = np.concatenate([res.results[i]["out_loc"] for i in range(cfg.NCORES)], axis=0)
    return out.astype(np.float32)


def kernel(x, lap, weight, bias, edge_index, num_nodes=None, **_kw):
    cfg = CFG(N=int(np.asarray(x).shape[0]), ncores=8)
    lap = np.asarray(lap, dtype=np.float32)
    edge_index = np.asarray(edge_index)
    meta, nc = _get_compiled(cfg, edge_index, lap)
    return _run(cfg, nc, meta, x, lap, weight, bias)
